# revision 1
# baseline (speedup 1.0000x reference)
"""Butterfly rotation (10 stages, DIM=1024) on 8 Trainium2 NeuronCores.

Math: each row x[n, :] undergoes 10 butterfly rotation stages; the whole
transform is linear.  Stages 0..7 (strides 1..128) only mix elements within
256-wide blocks, so their composite is block-diagonal with four dense
256x256 blocks (precomputed on host from `angles`).  Stages 8 and 9
(strides 256/512) pair whole 128-dim chunks and are applied on-chip as
per-element rotations with per-partition cos/sin scalars.

Device layout (per core, rows sharded 8192/core; pure data parallelism):
  - host pre-transposes each core's shard to dim-major tiles
    xin[g, p, c*512 + r] = x[g*512 + r, c*128 + p]   (g: 16 row-groups,
    c: 8 dim-chunks of 128, p: partition = dim-within-chunk, r: row)
  - PE: per group, 16 fp32 matmuls [K=128, M=128, N=512] (2 accumulating
    per output chunk) compute the stages-0..7 result in PSUM.
  - stage 8 (chunk pairs (0,2),(1,3),(4,6),(5,7)) evicts PSUM -> SBUF:
    ACT does the cross-term pre-scales, DVE the fused multiply-adds
    (scalar_tensor_tensor).  stage 9 (pairs (c, c+4)) repeats this
    SBUF -> SBUF into the output tile; each finished 1 MiB slice is
    DMA-stored immediately (stores on the ACT HWDGE ring, loads on SP).
  - host inverse-permutes the output tiles.

Empirical note: fp32 [128,128,512] matmuls measure ~1.05 us each here
(cold-clock + fp32 stream rate), so minimizing matmul count (16/group via
the 0..7 split, vs 32 for 0..8 or 64 with stage-9 folded into weights)
is what gets the kernel under the DMA roofline.
"""

import os
import sys

sys.path.insert(0, "/opt/trn_rl_repo")

# run_bass_kernel_spmd would try to import the (absent) axon NTFF hook if
# BASS_TRACE is set in the environment.
os.environ["BASS_NEVER_TRACE"] = "1"

import numpy as np

DIM = 1024
STAGES = 10
N_CORES = 8
ROWS_PER_CORE = 8192
GROUP_ROWS = 512
N_GROUPS = ROWS_PER_CORE // GROUP_ROWS  # 16
F32 = None  # set after bass import


def _stage_idx(dim, stage):
    stride = 2**stage
    idx_i = np.arange(dim).reshape(-1, 2 * stride)[:, :stride].ravel()
    idx_j = idx_i + stride
    return idx_i, idx_j


def _butterfly_apply(v, angles, stages):
    """Apply butterfly stages to rows of v (float64, in place) and return v."""
    for s in stages:
        idx_i, idx_j = _stage_idx(v.shape[1], s)
        c = np.cos(angles[s].astype(np.float64))
        sn = np.sin(angles[s].astype(np.float64))
        vi = v[:, idx_i].copy()
        vj = v[:, idx_j].copy()
        v[:, idx_i] = c * vi - sn * vj
        v[:, idx_j] = sn * vi + c * vj
    return v


def _host_tables(angles):
    """Stages 0..7 (strides 1..128) mix only within 256-wide blocks: their
    composite is block-diagonal with four dense 256x256 blocks B_q.  Stages 8
    and 9 are applied on-chip as per-element rotations between dim-chunks.

    wts[k, b, m], b = c*2 + t: lhsT block for output chunk c (0..7), input
    chunk ci = 2*(c//2) + t; block = B_{c//2}[t*128:(t+1)*128, (c%2)*128:...].

    trig[m, :]: stage-8 tables per chunk-pair (pairs (0,2),(1,3),(4,6),(5,7),
    first-chunk angle offsets [0,128,256,384]), then stage-9 per cg:
      cols 0:4   cos8[pidx], 4:8 sin8, 8:12 -sin8,
      cols 12:16 cos9[cg],  16:20 sin9, 20:24 -sin9.
    """
    mb = _butterfly_apply(np.eye(DIM, dtype=np.float64), angles, range(STAGES - 2))
    wts = np.empty((128, 16, 128), dtype=np.float32)
    for c in range(8):
        q = c // 2
        blk = mb[q * 256 : (q + 1) * 256, q * 256 : (q + 1) * 256]
        jl = (c % 2) * 128
        for t in range(2):
            wts[:, c * 2 + t, :] = blk[t * 128 : (t + 1) * 128, jl : jl + 128].astype(
                np.float32
            )
    # off-block-diagonal must vanish for stages 0..7
    mask = np.ones((DIM, DIM), dtype=bool)
    for q in range(4):
        mask[q * 256 : (q + 1) * 256, q * 256 : (q + 1) * 256] = False
    assert abs(mb[mask]).max() == 0.0

    th8 = angles[8].astype(np.float64)
    th9 = angles[9].astype(np.float64)
    trig = np.empty((128, 24), dtype=np.float32)
    for pidx, off in enumerate([0, 128, 256, 384]):
        sl = slice(off, off + 128)
        trig[:, pidx] = np.cos(th8[sl])
        trig[:, 4 + pidx] = np.sin(th8[sl])
        trig[:, 8 + pidx] = -np.sin(th8[sl])
    for cg in range(4):
        sl = slice(cg * 128, (cg + 1) * 128)
        trig[:, 12 + cg] = np.cos(th9[sl])
        trig[:, 16 + cg] = np.sin(th9[sl])
        trig[:, 20 + cg] = -np.sin(th9[sl])
    return wts, trig


def _pack_x(x_core, n_groups=N_GROUPS):
    # [G*512, 1024] -> [G, 128, 4096] with xin[g, p, c*512+r] = x[g*512+r, c*128+p]
    g = x_core.reshape(n_groups, GROUP_ROWS, 8, 128)
    return np.ascontiguousarray(g.transpose(0, 3, 2, 1).reshape(n_groups, 128, 4096))


def _unpack_y(y_packed, n_groups=N_GROUPS):
    # yout[g, p, cg*1024 + h*512 + r] = y[g*512 + r, (h*4 + cg)*128 + p]
    g = y_packed.reshape(n_groups, 128, 4, 2, GROUP_ROWS)
    return np.ascontiguousarray(
        g.transpose(0, 4, 3, 2, 1).reshape(n_groups * GROUP_ROWS, DIM)
    )


def _patch_tile_drain():
    """Workaround: this walrus build cannot encode semaphore waits on a
    sequencer Drain/NoOp with >1 wait ("Too many sync wait commands").
    Re-emit the TileContext tail waits as one nop per semaphore."""
    from concourse import mybir, tile
    from concourse.vector_clock import ScopedClock

    if getattr(tile.TileContext, "_drain_patched", False):
        return

    def _drain_and_barrier(self, tick_clock, wait_clock):
        nop_inst = self.nc.sync.nop(nofuse=True)
        wait_clock.add_sem_waits(
            nop_inst.ins, ScopedClock({None: tick_clock.global_clock})
        )
        si = nop_inst.ins.sync_info
        if si is not None and si.on_wait and len(si.on_wait) > 1:
            extra = si.on_wait[1:]
            si.on_wait = si.on_wait[:1]
            for w in extra:
                extra_nop = self.nc.sync.nop(nofuse=True)
                esi = extra_nop.ins.sync_info
                if esi is None:
                    extra_nop.ins.sync_info = mybir.SyncInfo(on_wait=[w], on_update=[])
                else:
                    esi.on_wait = list(esi.on_wait or []) + [w]
        self.nc.sync.drain()
        self.nc.all_engine_barrier()
        assert self.sems is not None
        popped = self.nc._tile_sem_poison_stack.pop()
        assert popped is self._sem_poison
        self.nc.clear_and_free_semaphores(list(self.sems.allocated().values()))
        self.nc.all_engine_barrier()

    tile.TileContext._drain_and_barrier = _drain_and_barrier
    tile.TileContext._drain_patched = True


def _split_multi_waits(nc, limit=1):
    """This walrus build encodes at most `limit` semaphore wait(s) per
    instruction ("Too many sync wait commands").  Hoist excess waits onto
    same-engine NoOps inserted immediately before the instruction."""
    from concourse import mybir

    counter = [0]

    def fresh_nop(engine, waits):
        counter[0] += 1
        nop = mybir.InstNoOp(
            name=f"waitsplit-{counter[0]}",
            engine=engine,
            ins=[],
            outs=[],
            bass_nofuse=True,
            sync_info=mybir.SyncInfo(on_wait=list(waits), on_update=[]),
        )
        nc.register_instruction(nop, overwrite=True)
        return nop

    for fn in nc.m.functions:
        for bb in fn.blocks:
            changed = False
            new = []
            for inst in bb.instructions:
                si = getattr(inst, "sync_info", None)
                if si is not None and si.on_wait and len(si.on_wait) > limit:
                    extra = si.on_wait[: len(si.on_wait) - limit]
                    si.on_wait = si.on_wait[len(si.on_wait) - limit :]
                    for k in range(0, len(extra), limit):
                        new.append(fresh_nop(inst.engine, extra[k : k + limit]))
                    changed = True
                new.append(inst)
            if changed:
                bb.instructions = new


def build_bass(n_groups=N_GROUPS, reps=1):
    """Build the Bass module for one core processing n_groups row-groups.
    reps>1 repeats the whole pipeline in-NEFF (for timing calibration)."""
    _patch_tile_drain()
    from concourse import bass, mybir, tile

    f32 = mybir.dt.float32
    nc = bass.Bass("TRN2", target_bir_lowering=False, debug=False)
    xin = nc.dram_tensor("xin", [n_groups, 128, 4096], f32, kind="ExternalInput")
    wts = nc.dram_tensor("wts", [128, 16, 128], f32, kind="ExternalInput")
    trig = nc.dram_tensor("trig", [128, 24], f32, kind="ExternalInput")
    yout = nc.dram_tensor("yout", [n_groups, 128, 4096], f32, kind="ExternalOutput")

    mult = mybir.AluOpType.mult
    add = mybir.AluOpType.add
    copy_fn = mybir.ActivationFunctionType.Copy

    with tile.TileContext(nc) as tc:
        with (
            tc.tile_pool(name="wp", bufs=1) as wp,
            tc.tile_pool(name="xp", bufs=3) as xp,
            tc.tile_pool(name="yp", bufs=3) as yp,
            tc.tile_pool(name="sp", bufs=3) as stp,
            tc.tile_pool(name="tp", bufs=6) as tp,
            tc.tile_pool(name="ps", bufs=8, space="PSUM") as psp,
        ):
            wt = wp.tile([128, 16, 128], f32)
            nc.sync.dma_start(wt[:], wts.ap()[:])
            tg = wp.tile([128, 24], f32)
            nc.sync.dma_start(tg[:], trig.ap()[:])

            for g in [g for _ in range(reps) for g in range(n_groups)]:
                xt = xp.tile([128, 4096], f32)
                nc.sync.dma_start(xt[:, 0:2048], xin.ap()[g][:, 0:2048])
                nc.sync.dma_start(xt[:, 2048:4096], xin.ap()[g][:, 2048:4096])
                yt = yp.tile([128, 4096], f32)
                st = stp.tile([128, 4096], f32)
                # per half: 8 matmuls (stages 0..7), then stage 8 in-half
                for h in range(2):
                    ps = [None] * 4
                    for lc in (0, 2, 1, 3):  # pair (0,2) completes first
                        c = h * 4 + lc
                        p = psp.tile([128, 512], f32, tag="ps")
                        for t in range(2):
                            ci = 2 * (c // 2) + t
                            nc.tensor.matmul(
                                p[:],
                                wt[:, c * 2 + t, :],
                                xt[:, ci * 512 : (ci + 1) * 512],
                                start=(t == 0),
                                stop=(t == 1),
                            )
                        ps[lc] = p
                    # stage 8 pairs within this half: (a, b) = (h*4, h*4+2), (h*4+1, h*4+3)
                    for k in range(2):
                        a, b = h * 4 + k, h * 4 + k + 2
                        pidx = h * 2 + k
                        pa, pb = ps[k], ps[k + 2]
                        t1 = tp.tile([128, 512], f32, tag="t")
                        nc.scalar.activation(
                            t1[:], pb[:], copy_fn, scale=tg[:, 8 + pidx : 9 + pidx]
                        )
                        nc.vector.scalar_tensor_tensor(
                            st[:, a * 512 : (a + 1) * 512],
                            pa[:], tg[:, pidx : pidx + 1], t1[:], mult, add,
                        )
                        t2 = tp.tile([128, 512], f32, tag="t")
                        nc.scalar.activation(
                            t2[:], pb[:], copy_fn, scale=tg[:, pidx : pidx + 1]
                        )
                        nc.vector.scalar_tensor_tensor(
                            st[:, b * 512 : (b + 1) * 512],
                            pa[:], tg[:, 4 + pidx : 5 + pidx], t2[:], mult, add,
                        )
                # stage 9 pairs across halves: (cg, cg+4)
                for cg in range(4):
                    sa = st[:, cg * 512 : (cg + 1) * 512]
                    sb = st[:, (cg + 4) * 512 : (cg + 5) * 512]
                    t3 = tp.tile([128, 512], f32, tag="t")
                    nc.scalar.activation(
                        t3[:], sb[:], copy_fn, scale=tg[:, 20 + cg : 21 + cg]
                    )
                    nc.vector.scalar_tensor_tensor(
                        yt[:, cg * 1024 : cg * 1024 + 512],
                        sa, tg[:, 12 + cg : 13 + cg], t3[:], mult, add,
                    )
                    t4 = tp.tile([128, 512], f32, tag="t")
                    nc.scalar.activation(
                        t4[:], sb[:], copy_fn, scale=tg[:, 12 + cg : 13 + cg]
                    )
                    nc.vector.scalar_tensor_tensor(
                        yt[:, cg * 1024 + 512 : (cg + 1) * 1024],
                        sa, tg[:, 16 + cg : 17 + cg], t4[:], mult, add,
                    )
                    nc.scalar.dma_start(
                        yout.ap()[g][:, cg * 1024 : (cg + 1) * 1024],
                        yt[:, cg * 1024 : (cg + 1) * 1024],
                    )
    _split_multi_waits(nc)
    return nc


_CACHE = {}


def _get_nc(n_groups=N_GROUPS):
    if n_groups not in _CACHE:
        _CACHE[n_groups] = build_bass(n_groups)
    return _CACHE[n_groups]


def make_in_maps(x, angles):
    """Pack full inputs into per-core in_maps (list of dicts)."""
    x = np.asarray(x, dtype=np.float32)
    angles = np.asarray(angles, dtype=np.float32)
    wts, trig = _host_tables(angles)
    flat = x.reshape(-1, DIM)
    in_maps = []
    for k in range(N_CORES):
        shard = flat[k * ROWS_PER_CORE : (k + 1) * ROWS_PER_CORE]
        in_maps.append({"xin": _pack_x(shard), "wts": wts, "trig": trig})
    return in_maps


def kernel(x, angles):
    from concourse.bass_utils import run_bass_kernel_spmd

    x = np.asarray(x)
    orig_shape = x.shape
    in_maps = make_in_maps(x, angles)
    nc = _get_nc()
    res = run_bass_kernel_spmd(nc, in_maps, core_ids=list(range(N_CORES)))
    parts = [_unpack_y(res.results[k]["yout"]) for k in range(N_CORES)]
    out = np.concatenate(parts, axis=0).reshape(orig_shape)
    return out.astype(np.float32)



# revision 2
# speedup vs baseline: 2.1230x; 2.1230x over previous
"""Butterfly rotation (10 stages, DIM=1024) on 8 Trainium2 NeuronCores.

Math: each row x[n, :] undergoes 10 butterfly rotation stages; the whole
transform is linear.  Stages 0..8 (strides 1..256) only mix elements within
512-wide blocks, so their composite is block-diagonal with two dense
512x512 blocks (precomputed on host from `angles`).  Stage 9 (stride 512)
pairs whole 128-dim chunks and is applied on-chip as per-element rotations
with per-partition cos/sin scalars.

Device layout (per core, rows sharded 8192/core; pure data parallelism):
  - host pre-transposes each core's shard to dim-major fp16 tiles
    xin[g, p, c*1024 + r] = x[g*1024 + r, c*128 + p]   (g: 8 row-groups,
    c: 8 dim-chunks of 128, p: partition = dim-within-chunk, r: row)
  - PE: per group, 4 chunk-pairs (c, c+4); per pair 16 fp16 matmuls
    [K=128, M=128, N=512] (4 accumulating per output 512-slice, K=512
    total) compute the stages-0..8 result in PSUM fp32.
  - stage 9 (pairs (c, c+4)) evicts PSUM -> SBUF fp16: ACT does the
    cross-term pre-scales, DVE the fused multiply-adds
    (scalar_tensor_tensor).  The finished group tile (2 MiB fp16) is
    DMA-stored once (stores on the ACT HWDGE ring, loads on SP).
  - host inverse-permutes + upcasts the output tiles.

Rationale: fp16 matmuls run ~4x faster than fp32 on the PE (N cycles at
2.4 GHz warm), so folding stage 8 into the weights (2x matmul count vs
stages 0..7) still beats paying for stage-8 pointwise ops on ACT/DVE,
which would otherwise be the bottleneck (ACT costs (N+352)/1.2GHz per op
regardless of dtype).  fp16 I/O halves DMA traffic to 32 MiB/core
(~94 us at the ~358 GB/s HBM-per-core limit); PE does 512 MM x ~216 ns
= ~111 us, which is the expected bottleneck.
"""

import os
import sys

sys.path.insert(0, "/opt/trn_rl_repo")

# run_bass_kernel_spmd would try to import the (absent) axon NTFF hook if
# BASS_TRACE is set in the environment.
os.environ["BASS_NEVER_TRACE"] = "1"

import numpy as np

DIM = 1024
STAGES = 10
N_CORES = 8
ROWS_PER_CORE = 8192
GROUP_ROWS = 1024
N_GROUPS = ROWS_PER_CORE // GROUP_ROWS  # 8


def _stage_idx(dim, stage):
    stride = 2**stage
    idx_i = np.arange(dim).reshape(-1, 2 * stride)[:, :stride].ravel()
    idx_j = idx_i + stride
    return idx_i, idx_j


def _butterfly_apply(v, angles, stages):
    """Apply butterfly stages to rows of v (float64, in place) and return v."""
    for s in stages:
        idx_i, idx_j = _stage_idx(v.shape[1], s)
        c = np.cos(angles[s].astype(np.float64))
        sn = np.sin(angles[s].astype(np.float64))
        vi = v[:, idx_i].copy()
        vj = v[:, idx_j].copy()
        v[:, idx_i] = c * vi - sn * vj
        v[:, idx_j] = sn * vi + c * vj
    return v


def _host_tables(angles):
    """Stages 0..8 (strides 1..256) mix only within 512-wide blocks: their
    composite is block-diagonal with two dense 512x512 blocks.  Stage 9 is
    applied on-chip as per-element rotations between chunk pairs (c, c+4).

    The identity transformed by _butterfly_apply yields mb with
    mb[input, output] (each row of eye is a basis vector).

    wts[k, b, m], b = c*4 + t: lhsT block for output chunk c (0..7), input
    chunk ci = 4*(c//4) + t; lhsT[k, m] = mb[ci*128 + k, c*128 + m].

    trig[p, :]: stage-9 tables per chunk pair c in 0..3 (angle slice
    th9[c*128:(c+1)*128]): cols 0:4 cos9, 4:8 sin9, 8:12 -sin9.
    """
    mb = _butterfly_apply(np.eye(DIM, dtype=np.float64), angles, range(STAGES - 1))
    # off-block-diagonal must vanish for stages 0..8
    mask = np.ones((DIM, DIM), dtype=bool)
    for q in range(2):
        mask[q * 512 : (q + 1) * 512, q * 512 : (q + 1) * 512] = False
    assert abs(mb[mask]).max() == 0.0

    wts = np.empty((128, 32, 128), dtype=np.float16)
    for c in range(8):
        for t in range(4):
            ci = 4 * (c // 4) + t
            wts[:, c * 4 + t, :] = mb[
                ci * 128 : (ci + 1) * 128, c * 128 : (c + 1) * 128
            ].astype(np.float16)

    th9 = angles[9].astype(np.float64)
    trig = np.empty((128, 12), dtype=np.float32)
    for c in range(4):
        sl = slice(c * 128, (c + 1) * 128)
        trig[:, c] = np.cos(th9[sl])
        trig[:, 4 + c] = np.sin(th9[sl])
        trig[:, 8 + c] = -np.sin(th9[sl])
    return wts, trig


def _pack_x(x_core, n_groups=N_GROUPS):
    # [G*1024, 1024] -> [G, 128, 8192] with xin[g, p, c*1024+r] = x[g*1024+r, c*128+p]
    g = x_core.astype(np.float16).reshape(n_groups, GROUP_ROWS, 8, 128)
    return np.ascontiguousarray(
        g.transpose(0, 3, 2, 1).reshape(n_groups, 128, 8 * GROUP_ROWS)
    )


def _unpack_y(y_packed, n_groups=N_GROUPS):
    # yout[g, p, c*1024 + r] = y[g*1024 + r, c*128 + p]  (inverse of _pack_x)
    g = y_packed.reshape(n_groups, 128, 8, GROUP_ROWS)
    return np.ascontiguousarray(
        g.transpose(0, 3, 2, 1).reshape(n_groups * GROUP_ROWS, DIM)
    ).astype(np.float32)


def _patch_tile_drain():
    """Workaround: this walrus build cannot encode semaphore waits on a
    sequencer Drain/NoOp with >1 wait ("Too many sync wait commands").
    Re-emit the TileContext tail waits as one nop per semaphore."""
    from concourse import mybir, tile
    from concourse.vector_clock import ScopedClock

    if getattr(tile.TileContext, "_drain_patched", False):
        return

    def _drain_and_barrier(self, tick_clock, wait_clock):
        nop_inst = self.nc.sync.nop(nofuse=True)
        wait_clock.add_sem_waits(
            nop_inst.ins, ScopedClock({None: tick_clock.global_clock})
        )
        si = nop_inst.ins.sync_info
        if si is not None and si.on_wait and len(si.on_wait) > 1:
            extra = si.on_wait[1:]
            si.on_wait = si.on_wait[:1]
            for w in extra:
                extra_nop = self.nc.sync.nop(nofuse=True)
                esi = extra_nop.ins.sync_info
                if esi is None:
                    extra_nop.ins.sync_info = mybir.SyncInfo(on_wait=[w], on_update=[])
                else:
                    esi.on_wait = list(esi.on_wait or []) + [w]
        self.nc.sync.drain()
        self.nc.all_engine_barrier()
        assert self.sems is not None
        popped = self.nc._tile_sem_poison_stack.pop()
        assert popped is self._sem_poison
        self.nc.clear_and_free_semaphores(list(self.sems.allocated().values()))
        self.nc.all_engine_barrier()

    tile.TileContext._drain_and_barrier = _drain_and_barrier
    tile.TileContext._drain_patched = True


def _split_multi_waits(nc, limit=1):
    """This walrus build encodes at most `limit` semaphore wait(s) per
    instruction ("Too many sync wait commands").  Hoist excess waits onto
    same-engine NoOps inserted immediately before the instruction."""
    from concourse import mybir

    counter = [0]

    def fresh_nop(engine, waits):
        counter[0] += 1
        nop = mybir.InstNoOp(
            name=f"waitsplit-{counter[0]}",
            engine=engine,
            ins=[],
            outs=[],
            bass_nofuse=True,
            sync_info=mybir.SyncInfo(on_wait=list(waits), on_update=[]),
        )
        nc.register_instruction(nop, overwrite=True)
        return nop

    for fn in nc.m.functions:
        for bb in fn.blocks:
            changed = False
            new = []
            for inst in bb.instructions:
                si = getattr(inst, "sync_info", None)
                if si is not None and si.on_wait and len(si.on_wait) > limit:
                    extra = si.on_wait[: len(si.on_wait) - limit]
                    si.on_wait = si.on_wait[len(si.on_wait) - limit :]
                    for k in range(0, len(extra), limit):
                        new.append(fresh_nop(inst.engine, extra[k : k + limit]))
                    changed = True
                new.append(inst)
            if changed:
                bb.instructions = new


def build_bass(n_groups=N_GROUPS, reps=1):
    """Build the Bass module for one core processing n_groups row-groups.
    reps>1 repeats the whole pipeline in-NEFF (for timing calibration)."""
    _patch_tile_drain()
    from concourse import bass, mybir, tile

    f32 = mybir.dt.float32
    f16 = mybir.dt.float16
    W = 8 * GROUP_ROWS  # 8192 columns per group tile
    nc = bass.Bass("TRN2", target_bir_lowering=False, debug=False)
    xin = nc.dram_tensor("xin", [n_groups, 128, W], f16, kind="ExternalInput")
    wts = nc.dram_tensor("wts", [128, 32, 128], f16, kind="ExternalInput")
    trig = nc.dram_tensor("trig", [128, 12], f32, kind="ExternalInput")
    yout = nc.dram_tensor("yout", [n_groups, 128, W], f16, kind="ExternalOutput")

    mult = mybir.AluOpType.mult
    add = mybir.AluOpType.add
    copy_fn = mybir.ActivationFunctionType.Copy

    with tile.TileContext(nc) as tc:
        with (
            tc.tile_pool(name="wp", bufs=1) as wp,
            tc.tile_pool(name="xp", bufs=3) as xp,
            tc.tile_pool(name="yp", bufs=3) as yp,
            tc.tile_pool(name="tp", bufs=4) as tp,
            tc.tile_pool(name="ps", bufs=2, space="PSUM") as psp,
        ):
            wt = wp.tile([128, 32, 128], f16)
            nc.sync.dma_start(wt[:], wts.ap()[:])
            tg = wp.tile([128, 12], f32)
            nc.sync.dma_start(tg[:], trig.ap()[:])

            for g in [g for _ in range(reps) for g in range(n_groups)]:
                xt = xp.tile([128, W], f16)
                nc.sync.dma_start(xt[:, 0 : W // 2], xin.ap()[g][:, 0 : W // 2])
                nc.sync.dma_start(xt[:, W // 2 : W], xin.ap()[g][:, W // 2 : W])
                yt = yp.tile([128, W], f16)
                for c in range(4):  # stage-9 pair (c, c+4)
                    pc = psp.tile([128, GROUP_ROWS], f32, tag="pc")
                    pc4 = psp.tile([128, GROUP_ROWS], f32, tag="pc4")
                    for dst, cc in ((pc, c), (pc4, c + 4)):
                        base = 4 * (cc // 4)
                        for nh in range(2):
                            o = nh * 512
                            for t in range(4):
                                col = (base + t) * GROUP_ROWS + o
                                nc.tensor.matmul(
                                    dst[:, o : o + 512],
                                    wt[:, cc * 4 + t, :],
                                    xt[:, col : col + 512],
                                    start=(t == 0),
                                    stop=(t == 3),
                                )
                    # stage 9: y_c = cos*pc - sin*pc4 ; y_{c+4} = sin*pc + cos*pc4
                    t1 = tp.tile([128, GROUP_ROWS], f16, tag="t")
                    nc.scalar.activation(
                        t1[:], pc4[:], copy_fn, scale=tg[:, 8 + c : 9 + c]
                    )
                    nc.vector.scalar_tensor_tensor(
                        yt[:, c * GROUP_ROWS : (c + 1) * GROUP_ROWS],
                        pc[:], tg[:, c : c + 1], t1[:], mult, add,
                    )
                    t2 = tp.tile([128, GROUP_ROWS], f16, tag="t")
                    nc.scalar.activation(t2[:], pc4[:], copy_fn, scale=tg[:, c : c + 1])
                    nc.vector.scalar_tensor_tensor(
                        yt[:, (c + 4) * GROUP_ROWS : (c + 5) * GROUP_ROWS],
                        pc[:], tg[:, 4 + c : 5 + c], t2[:], mult, add,
                    )
                nc.scalar.dma_start(yout.ap()[g][:], yt[:])
    _split_multi_waits(nc)
    return nc


_CACHE = {}


def _get_nc(n_groups=N_GROUPS):
    if n_groups not in _CACHE:
        _CACHE[n_groups] = build_bass(n_groups)
    return _CACHE[n_groups]


def make_in_maps(x, angles):
    """Pack full inputs into per-core in_maps (list of dicts)."""
    x = np.asarray(x, dtype=np.float32)
    angles = np.asarray(angles, dtype=np.float32)
    wts, trig = _host_tables(angles)
    flat = x.reshape(-1, DIM)
    in_maps = []
    for k in range(N_CORES):
        shard = flat[k * ROWS_PER_CORE : (k + 1) * ROWS_PER_CORE]
        in_maps.append({"xin": _pack_x(shard), "wts": wts, "trig": trig})
    return in_maps


def kernel(x, angles):
    from concourse.bass_utils import run_bass_kernel_spmd

    x = np.asarray(x)
    orig_shape = x.shape
    in_maps = make_in_maps(x, angles)
    nc = _get_nc()
    res = run_bass_kernel_spmd(nc, in_maps, core_ids=list(range(N_CORES)))
    parts = [_unpack_y(res.results[k]["yout"]) for k in range(N_CORES)]
    out = np.concatenate(parts, axis=0).reshape(orig_shape)
    return out.astype(np.float32)


# revision 4
# speedup vs baseline: 2.6990x; 1.2713x over previous
"""Butterfly rotation (10 stages, DIM=1024) on 8 Trainium2 NeuronCores.

Math: each row x[n, :] undergoes 10 butterfly rotation stages; the whole
transform is linear.  Stages 0..8 (strides 1..256) only mix elements within
512-wide blocks, so their composite is block-diagonal with two dense
512x512 blocks (precomputed on host from `angles`).  Stage 9 (stride 512)
pairs whole 128-dim chunks and is applied on-chip as per-element rotations
with per-partition cos/sin scalars.

Device layout (per core, rows sharded 8192/core; pure data parallelism):
  - host pre-transposes each core's shard to dim-major fp16 tiles
    xin[g, p, c*1024 + r] = x[g*1024 + r, c*128 + p]   (g: 8 row-groups,
    c: 8 dim-chunks of 128, p: partition = dim-within-chunk, r: row)
  - PE (fp16 matmuls [K=128, M=128, N=512], fp32 PSUM accum): chunks
    {0,2,4,5,6,7} get the full stages-0..8 result via K=512 (4
    accumulating matmuls per 512-slice); chunks {1,3} get only stages
    0..7 via K=256 (2 matmuls), and their stage 8 (pair (1,3)) is done
    pointwise on ACT/DVE.  This offload moves work from the bottleneck
    engine (PE: 56 instead of 64 matmuls/group) to engines with slack.
  - stage 8 pair (1,3) and stage 9 (pairs (c, c+4)) evict PSUM -> SBUF
    fp16: ACT does the cross-term pre-scales (reading the PSUM tile that
    finishes FIRST, so it overlaps the second tile's matmuls), DVE the
    fused multiply-adds (scalar_tensor_tensor).  The finished group tile
    (2 MiB fp16) is DMA-stored once (stores on ACT HWDGE ring, loads SP).
  - host inverse-permutes + upcasts the output tiles.

Engine budget per core (warm): PE 56*8 MM * ~216 ns ~ 97 us, ACT 80 ops
* ~1.15 us ~ 92 us, DVE 80 stt ~ 77 us, DMA 32 MiB / ~350 GB/s ~ 96 us.
"""

import os
import sys

sys.path.insert(0, "/opt/trn_rl_repo")

# run_bass_kernel_spmd would try to import the (absent) axon NTFF hook if
# BASS_TRACE is set in the environment.
os.environ["BASS_NEVER_TRACE"] = "1"

import numpy as np

DIM = 1024
STAGES = 10
N_CORES = 8
ROWS_PER_CORE = 8192
GROUP_ROWS = 1024
N_GROUPS = ROWS_PER_CORE // GROUP_ROWS  # 8


def _stage_idx(dim, stage):
    stride = 2**stage
    idx_i = np.arange(dim).reshape(-1, 2 * stride)[:, :stride].ravel()
    idx_j = idx_i + stride
    return idx_i, idx_j


def _butterfly_apply(v, angles, stages):
    """Apply butterfly stages to rows of v (float64, in place) and return v."""
    for s in stages:
        idx_i, idx_j = _stage_idx(v.shape[1], s)
        c = np.cos(angles[s].astype(np.float64))
        sn = np.sin(angles[s].astype(np.float64))
        vi = v[:, idx_i].copy()
        vj = v[:, idx_j].copy()
        v[:, idx_i] = c * vi - sn * vj
        v[:, idx_j] = sn * vi + c * vj
    return v


def _host_tables(angles):
    """wts[k, b, m] lhsT blocks (mb[input, output] transform of eye):
      b = c*4 + t, c in 0..7, t in 0..3: stages-0..8 block for output chunk
        c, input chunk 4*(c//4) + t (used for chunks {0,2,4,5,6,7});
      b = 32..35: stages-0..7 blocks for chunks 1, 3 (K=256: input chunks
        {0,1} for c=1, {2,3} for c=3).

    trig[p, :] fp32: cols c / 4+c / 8+c (c in 0..3): cos9_c / sin9_c /
    -sin9_c over th9[c*128:(c+1)*128]; cols 12/13/14: cos8 / sin8 / -sin8
    over th8[128:256] (the stage-8 pair (1,3) angles).
    """
    mb9 = _butterfly_apply(np.eye(DIM, dtype=np.float64), angles, range(9))
    mb8 = _butterfly_apply(np.eye(DIM, dtype=np.float64), angles, range(8))
    # stages 0..8 must be block-diagonal at 512; 0..7 at 256
    mask = np.ones((DIM, DIM), dtype=bool)
    for q in range(2):
        mask[q * 512 : (q + 1) * 512, q * 512 : (q + 1) * 512] = False
    assert abs(mb9[mask]).max() == 0.0
    mask = np.ones((DIM, DIM), dtype=bool)
    for q in range(4):
        mask[q * 256 : (q + 1) * 256, q * 256 : (q + 1) * 256] = False
    assert abs(mb8[mask]).max() == 0.0

    wts = np.zeros((128, 36, 128), dtype=np.float16)
    for c in range(8):
        for t in range(4):
            ci = 4 * (c // 4) + t
            wts[:, c * 4 + t, :] = mb9[
                ci * 128 : (ci + 1) * 128, c * 128 : (c + 1) * 128
            ].astype(np.float16)
    for i, c in enumerate((1, 3)):
        for t in range(2):
            ci = 2 * (c // 2) + t
            wts[:, 32 + i * 2 + t, :] = mb8[
                ci * 128 : (ci + 1) * 128, c * 128 : (c + 1) * 128
            ].astype(np.float16)

    th8 = angles[8].astype(np.float64)
    th9 = angles[9].astype(np.float64)
    trig = np.zeros((128, 16), dtype=np.float32)
    for c in range(4):
        sl = slice(c * 128, (c + 1) * 128)
        trig[:, c] = np.cos(th9[sl])
        trig[:, 4 + c] = np.sin(th9[sl])
        trig[:, 8 + c] = -np.sin(th9[sl])
    trig[:, 12] = np.cos(th8[128:256])
    trig[:, 13] = np.sin(th8[128:256])
    trig[:, 14] = -np.sin(th8[128:256])
    return wts, trig


def _pack_x(x_core, n_groups=N_GROUPS):
    # [G*1024, 1024] -> [G, 128, 8192] with xin[g, p, c*1024+r] = x[g*1024+r, c*128+p]
    g = x_core.astype(np.float16).reshape(n_groups, GROUP_ROWS, 8, 128)
    return np.ascontiguousarray(
        g.transpose(0, 3, 2, 1).reshape(n_groups, 128, 8 * GROUP_ROWS)
    )


def _unpack_y(y_packed, n_groups=N_GROUPS):
    # yout[g, p, c*1024 + r] = y[g*1024 + r, c*128 + p]  (inverse of _pack_x)
    g = y_packed.reshape(n_groups, 128, 8, GROUP_ROWS)
    return np.ascontiguousarray(
        g.transpose(0, 3, 2, 1).reshape(n_groups * GROUP_ROWS, DIM)
    ).astype(np.float32)


def _patch_tile_drain():
    """Workaround: this walrus build cannot encode semaphore waits on a
    sequencer Drain/NoOp with >1 wait ("Too many sync wait commands").
    Re-emit the TileContext tail waits as one nop per semaphore."""
    from concourse import mybir, tile
    from concourse.vector_clock import ScopedClock

    if getattr(tile.TileContext, "_drain_patched", False):
        return

    def _drain_and_barrier(self, tick_clock, wait_clock):
        nop_inst = self.nc.sync.nop(nofuse=True)
        wait_clock.add_sem_waits(
            nop_inst.ins, ScopedClock({None: tick_clock.global_clock})
        )
        si = nop_inst.ins.sync_info
        if si is not None and si.on_wait and len(si.on_wait) > 1:
            extra = si.on_wait[1:]
            si.on_wait = si.on_wait[:1]
            for w in extra:
                extra_nop = self.nc.sync.nop(nofuse=True)
                esi = extra_nop.ins.sync_info
                if esi is None:
                    extra_nop.ins.sync_info = mybir.SyncInfo(on_wait=[w], on_update=[])
                else:
                    esi.on_wait = list(esi.on_wait or []) + [w]
        self.nc.sync.drain()
        self.nc.all_engine_barrier()
        assert self.sems is not None
        popped = self.nc._tile_sem_poison_stack.pop()
        assert popped is self._sem_poison
        self.nc.clear_and_free_semaphores(list(self.sems.allocated().values()))
        self.nc.all_engine_barrier()

    tile.TileContext._drain_and_barrier = _drain_and_barrier
    tile.TileContext._drain_patched = True


def _split_multi_waits(nc, limit=1):
    """This walrus build encodes at most `limit` semaphore wait(s) per
    instruction ("Too many sync wait commands").  Hoist excess waits onto
    same-engine NoOps inserted immediately before the instruction."""
    from concourse import mybir

    counter = [0]

    def fresh_nop(engine, waits):
        counter[0] += 1
        nop = mybir.InstNoOp(
            name=f"waitsplit-{counter[0]}",
            engine=engine,
            ins=[],
            outs=[],
            bass_nofuse=True,
            sync_info=mybir.SyncInfo(on_wait=list(waits), on_update=[]),
        )
        nc.register_instruction(nop, overwrite=True)
        return nop

    for fn in nc.m.functions:
        for bb in fn.blocks:
            changed = False
            new = []
            for inst in bb.instructions:
                si = getattr(inst, "sync_info", None)
                if si is not None and si.on_wait and len(si.on_wait) > limit:
                    extra = si.on_wait[: len(si.on_wait) - limit]
                    si.on_wait = si.on_wait[len(si.on_wait) - limit :]
                    for k in range(0, len(extra), limit):
                        new.append(fresh_nop(inst.engine, extra[k : k + limit]))
                    changed = True
                new.append(inst)
            if changed:
                bb.instructions = new


def build_bass(n_groups=N_GROUPS, reps=1):
    """Build the Bass module for one core processing n_groups row-groups.
    reps>1 repeats the whole pipeline in-NEFF (for timing calibration)."""
    _patch_tile_drain()
    from concourse import bass, mybir, tile

    f32 = mybir.dt.float32
    f16 = mybir.dt.float16
    GR = GROUP_ROWS
    W = 8 * GR  # 8192 columns per group tile
    nc = bass.Bass("TRN2", target_bir_lowering=False, debug=False)
    xin = nc.dram_tensor("xin", [n_groups, 128, W], f16, kind="ExternalInput")
    wts = nc.dram_tensor("wts", [128, 36, 128], f16, kind="ExternalInput")
    trig = nc.dram_tensor("trig", [128, 16], f32, kind="ExternalInput")
    yout = nc.dram_tensor("yout", [n_groups, 128, W], f16, kind="ExternalOutput")

    mult = mybir.AluOpType.mult
    add = mybir.AluOpType.add
    copy_fn = mybir.ActivationFunctionType.Copy

    with tile.TileContext(nc) as tc:
        with (
            tc.tile_pool(name="wp", bufs=1) as wp,
            tc.tile_pool(name="xp", bufs=3) as xp,
            tc.tile_pool(name="yp", bufs=3) as yp,
            tc.tile_pool(name="tp", bufs=6) as tp,
            tc.tile_pool(name="s8p", bufs=4) as s8p,
            tc.tile_pool(name="ps", bufs=4, space="PSUM") as psp,
        ):
            wt = wp.tile([128, 36, 128], f16)
            nc.sync.dma_start(wt[:], wts.ap()[:])
            tg = wp.tile([128, 16], f32)
            nc.sync.dma_start(tg[:], trig.ap()[:])

            def mm_full(dst, c):
                """stages 0..8: K=512 over input chunks 4*(c//4)+t."""
                base = 4 * (c // 4)
                for nh in range(2):
                    o = nh * 512
                    for t in range(4):
                        col = (base + t) * GR + o
                        nc.tensor.matmul(
                            dst[:, o : o + 512],
                            wt[:, c * 4 + t, :],
                            xt[:, col : col + 512],
                            start=(t == 0),
                            stop=(t == 3),
                        )

            def mm_k256(dst, slot0, c):
                """stages 0..7: K=256 over input chunks 2*(c//2)+t."""
                base = 2 * (c // 2)
                for nh in range(2):
                    o = nh * 512
                    for t in range(2):
                        col = (base + t) * GR + o
                        nc.tensor.matmul(
                            dst[:, o : o + 512],
                            wt[:, slot0 + t, :],
                            xt[:, col : col + 512],
                            start=(t == 0),
                            stop=(t == 1),
                        )

            def scales(src, cols):
                """ACT pre-scales: fp16 tiles tg[:,col] * src for col in cols."""
                outs = []
                for col in cols:
                    tt = tp.tile([128, GR], f16, tag="t")
                    nc.scalar.activation(
                        tt[:], src[:], copy_fn, scale=tg[:, col : col + 1]
                    )
                    outs.append(tt)
                return outs

            def rotate(out_a, out_b, second, t1, t2, col_a, col_b):
                """out_a = t1 + col_a*second ; out_b = t2 + col_b*second
                (t1/t2 are ACT pre-scales of the pair's FIRST psum tile)."""
                nc.vector.scalar_tensor_tensor(
                    out_a, second[:], tg[:, col_a : col_a + 1], t1[:], mult, add
                )
                nc.vector.scalar_tensor_tensor(
                    out_b, second[:], tg[:, col_b : col_b + 1], t2[:], mult, add
                )

            for g in [g for _ in range(reps) for g in range(n_groups)]:
                xt = xp.tile([128, W], f16)
                nc.sync.dma_start(xt[:, 0 : W // 2], xin.ap()[g][:, 0 : W // 2])
                nc.sync.dma_start(xt[:, W // 2 : W], xin.ap()[g][:, W // 2 : W])
                yt = yp.tile([128, W], f16)

                def ysl(c):
                    return yt[:, c * GR : (c + 1) * GR]

                # round 1-2: stages 0..7 of chunks 1, 3; stage-8 pair (1,3)
                ps1 = psp.tile([128, GR], f32, tag="ps")
                mm_k256(ps1, 32, 1)
                ps3 = psp.tile([128, GR], f32, tag="ps")
                mm_k256(ps3, 34, 3)
                tA, tB = scales(ps1, (12, 13))  # cos8*s1, sin8*s1
                s8_1 = s8p.tile([128, GR], f16, tag="s8")
                s8_3 = s8p.tile([128, GR], f16, tag="s8")
                # s8_1 = -sin8*s3 + cos8*s1 ; s8_3 = cos8*s3 + sin8*s1
                rotate(s8_1[:], s8_3[:], ps3, tA, tB, 14, 12)

                # rounds 3-4: pair (0,4)
                p0 = psp.tile([128, GR], f32, tag="ps")
                mm_full(p0, 0)
                p4 = psp.tile([128, GR], f32, tag="ps")
                mm_full(p4, 4)
                t1, t2 = scales(p0, (0, 4))  # cos9_0*p0, sin9_0*p0
                rotate(ysl(0), ysl(4), p4, t1, t2, 8, 0)

                # round 5: pair (1,5) — s8_1 is fp16 in SBUF, p5 in PSUM
                p5 = psp.tile([128, GR], f32, tag="ps")
                mm_full(p5, 5)
                t1, t2 = scales(p5, (9, 1))  # -sin9_1*p5, cos9_1*p5
                rotate(ysl(1), ysl(5), s8_1, t1, t2, 1, 5)

                # rounds 6-7: pair (2,6)
                p2 = psp.tile([128, GR], f32, tag="ps")
                mm_full(p2, 2)
                p6 = psp.tile([128, GR], f32, tag="ps")
                mm_full(p6, 6)
                t1, t2 = scales(p2, (2, 6))
                rotate(ysl(2), ysl(6), p6, t1, t2, 10, 2)

                # round 8: pair (3,7)
                p7 = psp.tile([128, GR], f32, tag="ps")
                mm_full(p7, 7)
                t1, t2 = scales(p7, (11, 3))  # -sin9_3*p7, cos9_3*p7
                rotate(ysl(3), ysl(7), s8_3, t1, t2, 3, 7)

                nc.scalar.dma_start(yout.ap()[g][:], yt[:])
    _split_multi_waits(nc)
    return nc


_CACHE = {}


def _get_nc(n_groups=N_GROUPS):
    if n_groups not in _CACHE:
        _CACHE[n_groups] = build_bass(n_groups)
    return _CACHE[n_groups]


def make_in_maps(x, angles):
    """Pack full inputs into per-core in_maps (list of dicts)."""
    x = np.asarray(x, dtype=np.float32)
    angles = np.asarray(angles, dtype=np.float32)
    wts, trig = _host_tables(angles)
    flat = x.reshape(-1, DIM)
    in_maps = []
    for k in range(N_CORES):
        shard = flat[k * ROWS_PER_CORE : (k + 1) * ROWS_PER_CORE]
        in_maps.append({"xin": _pack_x(shard), "wts": wts, "trig": trig})
    return in_maps


def kernel(x, angles):
    from concourse.bass_utils import run_bass_kernel_spmd

    x = np.asarray(x)
    orig_shape = x.shape
    in_maps = make_in_maps(x, angles)
    nc = _get_nc()
    res = run_bass_kernel_spmd(nc, in_maps, core_ids=list(range(N_CORES)))
    parts = [_unpack_y(res.results[k]["yout"]) for k in range(N_CORES)]
    out = np.concatenate(parts, axis=0).reshape(orig_shape)
    return out.astype(np.float32)
